# revision 18
# baseline (speedup 1.0000x reference)
"""GNN message-passing kernel for 8 Trainium2 NeuronCores.

Math (see reference):
  out[e] = relu(BN_E(local[e] + global[e]))
  local[e]  = emb_src[feat[src_e]] @ We0 + emb_dst[feat[dst_e]] @ We1 + b_edge
  global[e] = (P1[src_e] @ P2[dst_e]) @ W3 + b3

Device strategy (edge-parallel, 40000 edges/core, 320 tiles of 128 edges):
  - Host folds W3 into W1 (W1f) and b1/b3 into a per-dst-node vector; builds
    two per-core COMPACTED bf16 node tables (int16 gather indices):
      SRCT[r] = [h[n] | emb_src-term[n] | pad]           (256B rows)
      DSTT[r] = [P2[n] | 1.0 | emb_dst-term+P2B[n] | pad] (256B rows)
  - src rows are fetched with TRANSPOSED dma_gather (512 idx/instr): the
    gather output IS the matmul lhsT — no on-device transposes at all.
  - dst rows with plain dma_gather (1024 idx/instr), row-major.
  - Per 128-edge tile:
      PE:  T1 = hsT.T @ W1f (bf16, 1024 cols)  [128, 1024] PSUM
           LS = esT.T @ I32 (recovers local-src row-major) [128, 32] PSUM
      DVE: z = T1 * bcast(P2_dst); g = reduce_d(z); g += LS; g += local-dst
  - BN stats computed at the end from `raw` (off the per-tile chain),
    AllReduce across 8 cores, scale/bias broadcast, normalize+relu pass.
"""

import os
import numpy as np

H = 32
N = 40000
E = 320000
NCORES = 8
EC = E // NCORES          # 40000 edges per core
SCH = 512                 # src edges per transposed dma_gather
DCH = 1024                # dst edges per plain dma_gather
NDC = 40                  # dst chunks per core
ECP = DCH * NDC           # 40960 padded edges per core
TPC = ECP // 128          # 320 tiles of 128 edges
PAD = ECP - EC            # 960 dummy edges per core
TCAP = 32768              # compacted node-table capacity (int16 index range)
EPS = 1e-5

_cache = {}
last_exec_time_ns = None
last_results = None


def _build():
    if "nc" in _cache:
        return _cache["nc"]
    repeat = int(os.environ.get("KERNEL_REPEAT", "1"))

    import concourse.bacc as bacc
    import concourse.bass as bass
    import concourse.mybir as mybir
    import concourse.tile as tile

    f32 = mybir.dt.float32
    bf16 = mybir.dt.bfloat16
    i16 = mybir.dt.int16
    AF = mybir.ActivationFunctionType
    OP = mybir.AluOpType

    nc = bacc.Bacc("TRN2", target_bir_lowering=False, debug=False,
                   num_devices=NCORES)

    SRCT = nc.dram_tensor("srct", [TCAP, 128], bf16, kind="ExternalInput").ap()
    DSTT = nc.dram_tensor("dstt", [TCAP, 128], bf16, kind="ExternalInput").ap()
    W1F = nc.dram_tensor("w1f", [H, H * H], bf16, kind="ExternalInput").ap()
    ID32 = nc.dram_tensor("id32", [2 * H, H], bf16, kind="ExternalInput").ap()
    SIDX = nc.dram_tensor("sidx", [128, ECP // 16], i16,
                          kind="ExternalInput").ap()
    DIDX = nc.dram_tensor("didx", [128, ECP // 16], i16,
                          kind="ExternalInput").ap()
    GB = nc.dram_tensor("gb", [1, 64], f32, kind="ExternalInput").ap()
    CORR = nc.dram_tensor("corr", [1, 64], f32, kind="ExternalInput").ap()
    OUT = nc.dram_tensor("out", [128, TPC * H], f32, kind="ExternalOutput").ap()

    OB = 32  # tiles per output write batch

    with tile.TileContext(nc) as tc:
        with tc.tile_pool(name="const", bufs=1) as cpool, \
             tc.tile_pool(name="big", bufs=1) as bigpool, \
             tc.tile_pool(name="gs", bufs=6) as gs_pool, \
             tc.tile_pool(name="gd", bufs=4) as gd_pool, \
             tc.tile_pool(name="work", bufs=2) as wpool, \
             tc.tile_pool(name="psA", bufs=2, space="PSUM") as psA, \
             tc.tile_pool(name="psB", bufs=2, space="PSUM") as psB, \
             tc.tile_pool(name="psmisc", bufs=1, space="PSUM") as psmisc, \
             tc.tile_pool(name="dram", bufs=1, space="DRAM") as dpool:

            w1f_s = cpool.tile([H, H * H], bf16)
            nc.sync.dma_start(w1f_s[:], W1F[:])
            id32_s = cpool.tile([2 * H, H], bf16)
            nc.sync.dma_start(id32_s[:], ID32[:])
            sidx_s = cpool.tile([128, ECP // 16], i16)
            nc.sync.dma_start(sidx_s[:], SIDX[:])
            didx_s = cpool.tile([128, ECP // 16], i16)
            nc.sync.dma_start(didx_s[:], DIDX[:])
            gb_s = cpool.tile([1, 64], f32)
            nc.sync.dma_start(gb_s[:], GB[:])
            corr_s = cpool.tile([1, 64], f32)
            nc.sync.dma_start(corr_s[:], CORR[:])
            ones_col = cpool.tile([128, 1], f32)
            nc.vector.memset(ones_col[:], 1.0)
            ones_row = cpool.tile([1, 128], f32)
            nc.vector.memset(ones_row[:], 1.0)

            raw = bigpool.tile([128, TPC * H], f32)       # raw pre-BN output

            # ---------------- pass 1: per-tile bilinear ----------------
            for rep in range(repeat):
              for dc in range(NDC):
                pdch = gd_pool.tile([128, DCH // 128, 128], bf16, tag="pd")
                nc.gpsimd.dma_gather(
                    pdch[:], DSTT[:], didx_s[:, dc * 64:dc * 64 + 64],
                    DCH, DCH, 128)
                for sh in range(2):
                    sc = dc * 2 + sh
                    hsxT = gs_pool.tile([128, 1, SCH], bf16, tag="hs")
                    nc.gpsimd.dma_gather(
                        hsxT[:], SRCT[:], sidx_s[:, sc * 32:sc * 32 + 32],
                        SCH, SCH, 128, transpose=True)
                    for u in range(4):
                        t = sc * 4 + u
                        cols = slice(u * 128, (u + 1) * 128)
                        v = sh * 4 + u      # tile within the dst chunk

                        t1 = psA.tile([128, H * H], f32, tag="t1")
                        nc.tensor.matmul(out=t1[:, 0:512],
                                         lhsT=hsxT[0:H, 0, cols],
                                         rhs=w1f_s[:, 0:512],
                                         start=True, stop=True)
                        nc.tensor.matmul(out=t1[:, 512:1024],
                                         lhsT=hsxT[0:H, 0, cols],
                                         rhs=w1f_s[:, 512:1024],
                                         start=True, stop=True)
                        ls = psB.tile([128, H], f32, tag="ls")
                        nc.tensor.matmul(out=ls[:],
                                         lhsT=hsxT[H:2 * H, 0, cols],
                                         rhs=id32_s[H:2 * H, :],
                                         start=True, stop=True)

                        z = wpool.tile([128, H * H], f32, tag="z")
                        pd_b = pdch[:, v, 0:H].unsqueeze(1).to_broadcast(
                            [128, H, H])
                        nc.vector.tensor_tensor(
                            out=z[:],
                            in0=t1[:].rearrange("p (m d) -> p m d", d=H),
                            in1=pd_b, op=OP.mult)

                        g = raw[:, t * H:(t + 1) * H]
                        nc.vector.tensor_reduce(
                            out=g, in_=z[:].rearrange("p (m d) -> p m d", d=H),
                            axis=mybir.AxisListType.X, op=OP.add)
                        nc.vector.tensor_tensor(out=g, in0=g, in1=ls[:],
                                                op=OP.add)
                        nc.vector.tensor_tensor(
                            out=g, in0=g, in1=pdch[:, v, H + 1:2 * H + 1],
                            op=OP.add)

            # ---------------- stats (end pass) + allreduce -------------
            ssacc = psmisc.tile([1, 64], f32, tag="ssacc")
            part = wpool.tile([128, H], f32, tag="part")
            nc.vector.tensor_reduce(
                out=part[:], in_=raw[:].rearrange("p (t m) -> p m t", m=H),
                axis=mybir.AxisListType.X, op=OP.add)
            nc.tensor.matmul(out=ssacc[:, 0:H], lhsT=ones_col[:],
                             rhs=part[:], start=True, stop=True,
                             skip_group_check=True)
            sqacc = wpool.tile([128, H], f32, tag="sqacc")
            NSQ = 4
            SQW = TPC * H // NSQ
            for j in range(NSQ):
                sqc = wpool.tile([128, SQW], f32, tag="sqc")
                nc.scalar.square(sqc[:], raw[:, j * SQW:(j + 1) * SQW])
                pj = wpool.tile([128, H], f32, tag="pj")
                nc.vector.tensor_reduce(
                    out=pj[:], in_=sqc[:].rearrange("p (t m) -> p m t", m=H),
                    axis=mybir.AxisListType.X, op=OP.add)
                if j == 0:
                    nc.scalar.copy(sqacc[:], pj[:])
                else:
                    nc.vector.tensor_tensor(out=sqacc[:], in0=sqacc[:],
                                            in1=pj[:], op=OP.add)
            nc.tensor.matmul(out=ssacc[:, H:2 * H], lhsT=ones_col[:],
                             rhs=sqacc[:], start=True, stop=True,
                             skip_group_check=True)

            stats = cpool.tile([1, 64], f32)
            nc.scalar.copy(stats[:], ssacc[:])
            gstats = cpool.tile([1, 64], f32)
            if os.environ.get("KERNEL_1CORE", "0") == "1":
                nc.scalar.copy(gstats[:], stats[:])
            else:
                cin = dpool.tile([1, 64], f32)
                cout = dpool.tile([1, 64], f32)
                nc.sync.dma_start(cin[:], stats[:])
                nc.gpsimd.collective_compute(
                    "AllReduce", OP.add,
                    replica_groups=[list(range(NCORES))],
                    ins=[cin.opt()], outs=[cout.opt()])
                nc.sync.dma_start(gstats[:], cout[:])

            mv = cpool.tile([1, 64], f32)
            nc.vector.tensor_tensor(out=mv[:], in0=gstats[:], in1=corr_s[:],
                                    op=OP.subtract)
            nc.vector.tensor_scalar_mul(mv[:], mv[:], 1.0 / E)
            var = cpool.tile([1, H], f32)
            nc.vector.tensor_tensor(out=var[:], in0=mv[:, 0:H],
                                    in1=mv[:, 0:H], op=OP.mult)
            nc.vector.tensor_tensor(out=var[:], in0=mv[:, H:2 * H],
                                    in1=var[:], op=OP.subtract)
            nc.vector.tensor_scalar_add(var[:], var[:], EPS)
            sd = cpool.tile([1, H], f32)
            nc.scalar.activation(sd[:], var[:], AF.Sqrt)
            rs = cpool.tile([1, H], f32)
            nc.vector.reciprocal(rs[:], sd[:])

            scaleb = cpool.tile([1, 64], f32)
            nc.vector.tensor_tensor(out=scaleb[:, 0:H], in0=gb_s[:, 0:H],
                                    in1=rs[:], op=OP.mult)
            tmp1 = cpool.tile([1, H], f32)
            nc.vector.tensor_tensor(out=tmp1[:], in0=mv[:, 0:H],
                                    in1=scaleb[:, 0:H], op=OP.mult)
            nc.vector.tensor_tensor(out=scaleb[:, H:2 * H], in0=gb_s[:, H:2 * H],
                                    in1=tmp1[:], op=OP.subtract)

            sb_p = psmisc.tile([128, 64], f32, tag="sbp")
            nc.tensor.matmul(out=sb_p[:], lhsT=ones_row[:], rhs=scaleb[:],
                             start=True, stop=True, skip_group_check=True)
            sb = cpool.tile([128, 64], f32)
            nc.scalar.copy(sb[:], sb_p[:])

            # ---------------- pass 2: normalize + relu -----------------
            sc_b = sb[:, 0:H].unsqueeze(1).to_broadcast([128, OB, H])
            bi_b = sb[:, H:2 * H].unsqueeze(1).to_broadcast([128, OB, H])
            for b0 in range(0, TPC, OB):
                ob = wpool.tile([128, OB, H], f32, tag="ob")
                rawv = raw[:, b0 * H:(b0 + OB) * H].rearrange(
                    "p (t m) -> p t m", m=H)
                nc.vector.tensor_tensor(out=ob[:], in0=rawv, in1=sc_b,
                                        op=OP.mult)
                nc.vector.tensor_tensor(out=ob[:], in0=ob[:], in1=bi_b,
                                        op=OP.add)
                nc.scalar.activation(ob[:], ob[:], AF.Relu)
                nc.sync.dma_start(
                    OUT[:, b0 * H:(b0 + OB) * H],
                    ob[:].rearrange("p t m -> p (t m)"))

    nc.compile()
    _cache["nc"] = nc
    return nc


def _run_sim(nc, in_maps):
    import numpy as np
    from concourse.bass_interp import MultiCoreSim
    from concourse import bass_utils, mybir

    sim = MultiCoreSim(nc, num_cores=NCORES, num_workers=NCORES)
    for c in range(NCORES):
        core = sim.cores[c]
        for name, val in in_maps[c].items():
            core.tensor(name)[:] = val
        if nc.partition_id_tensor is not None:
            core.tensor(nc.partition_id_tensor.name)[:] = np.array(
                [[c]], dtype=np.uint32)
    sim.simulate()
    results = []
    for c in range(NCORES):
        outs = {}
        for alloc in nc.m.functions[0].allocations:
            if isinstance(alloc, mybir.MemoryLocationSet) and \
                    alloc.kind == "ExternalOutput":
                name = alloc.memorylocations[0].name
                outs[name] = np.array(sim.cores[c].tensor(name))
        results.append(outs)
    return bass_utils.BassKernelResults(
        results=results, instructions_and_trace=None, profile_json=None,
        exec_time_ns=None)


def _prep_idx16(inv):
    """Wrap int16 indices into the dma_gather layout: idx i at
    [i % 16, i // 16], replicated across the 8 16-partition groups."""
    pad = np.zeros(ECP, dtype=np.int16)
    pad[:EC] = inv
    w = np.ascontiguousarray(pad.reshape(ECP // 16, 16).T)   # [16, ECP//16]
    return np.ascontiguousarray(np.tile(w, (8, 1)))          # [128, ECP//16]


def kernel(h, e, feat, src_idx, dst_idx, emb_src, emb_dst, W_edge, b_edge,
           W1, b1, W2, b2, W3, b3, gamma, beta):
    global last_exec_time_ns, last_results
    import ml_dtypes
    import concourse.bass_utils as bass_utils

    bf = ml_dtypes.bfloat16
    h = np.asarray(h, np.float32)
    feat = np.asarray(feat, np.int64)
    src_idx = np.asarray(src_idx, np.int64)
    dst_idx = np.asarray(dst_idx, np.int64)
    emb_src = np.asarray(emb_src, np.float32)
    emb_dst = np.asarray(emb_dst, np.float32)
    W_edge = np.asarray(W_edge, np.float32)
    b_edge = np.asarray(b_edge, np.float32)
    W1 = np.asarray(W1, np.float32)
    b1 = np.asarray(b1, np.float32)
    W2 = np.asarray(W2, np.float32)
    b2 = np.asarray(b2, np.float32)
    W3 = np.asarray(W3, np.float32)
    b3 = np.asarray(b3, np.float32)
    gamma = np.asarray(gamma, np.float32)
    beta = np.asarray(beta, np.float32)

    # ---- host-side weight folds and node tables ----
    ES = emb_src @ W_edge[:H] + 0.5 * b_edge              # [V, H]
    ED = emb_dst @ W_edge[H:] + 0.5 * b_edge
    W1r = W1.reshape(H, H, H)                             # [i, k, d]
    W1f = np.ascontiguousarray(
        np.einsum("ikd,km->imd", W1r, W3).reshape(H, H * H)).astype(np.float32)
    Btil = np.einsum("kd,km->dm", b1.reshape(H, H), W3)   # [d, m]
    P2 = h @ W2 + b2                                      # [N, H]
    P2B = P2 @ Btil + b3                                  # [N, H]
    Hcat = np.ascontiguousarray(
        np.concatenate([h, ES[feat]], axis=1)).astype(np.float32)
    Pcat = np.ascontiguousarray(
        np.concatenate([P2, ED[feat] + P2B], axis=1)).astype(np.float32)

    gb = np.concatenate([gamma, beta]).reshape(1, 64).astype(np.float32)
    W1fb = W1f.astype(bf)
    id32 = np.zeros((2 * H, H), np.float32)
    id32[H:2 * H] = np.eye(H, dtype=np.float32)
    id32 = id32.astype(bf)

    nc = _build()

    corr_sum = np.zeros(H, np.float64)
    corr_sq = np.zeros(H, np.float64)
    W1f3d = W1fb.astype(np.float64).reshape(H, H, H)      # [i, m, d]
    per_core = []
    for c in range(NCORES):
        sl = slice(c * EC, (c + 1) * EC)
        su, sinv = np.unique(src_idx[sl], return_inverse=True)
        du, dinv = np.unique(dst_idx[sl], return_inverse=True)
        assert len(su) <= TCAP and len(du) <= TCAP, (len(su), len(du))
        SRCTc = np.zeros((TCAP, 128), bf)
        SRCTc[:len(su), 0:H] = Hcat[su, 0:H].astype(bf)
        SRCTc[:len(su), H:2 * H] = Hcat[su, H:2 * H].astype(bf)
        DSTTc = np.zeros((TCAP, 128), bf)
        DSTTc[:len(du), 0:H] = Pcat[du, 0:H].astype(bf)
        DSTTc[:len(du), H] = np.float32(1.0)
        DSTTc[:len(du), H + 1:2 * H + 1] = Pcat[du, H:2 * H].astype(bf)
        per_core.append((SRCTc, DSTTc, sinv.astype(np.int16),
                         dinv.astype(np.int16)))
        # dummy padded edge (table rows 0, 0) contribution to BN stats
        hrow = SRCTc[0, 0:H].astype(np.float64)
        esrow = SRCTc[0, H:2 * H].astype(np.float64)
        prow = DSTTc[0, 0:H].astype(np.float64)
        ldrow = DSTTc[0, H + 1:2 * H + 1].astype(np.float64)
        v = np.einsum("i,imd,d->m", hrow, W1f3d, prow) + esrow + ldrow
        corr_sum += PAD * v
        corr_sq += PAD * v * v

    corr = np.zeros((1, 64), np.float32)
    corr[0, :H] = corr_sum
    corr[0, H:] = corr_sq

    in_maps = []
    for c in range(NCORES):
        SRCTc, DSTTc, sinv, dinv = per_core[c]
        in_maps.append({
            "srct": SRCTc,
            "dstt": DSTTc,
            "w1f": W1fb,
            "id32": id32,
            "sidx": _prep_idx16(sinv),
            "didx": _prep_idx16(dinv),
            "gb": gb,
            "corr": corr,
        })

    _cache["last_in_maps"] = in_maps
    if os.environ.get("KERNEL_SIM", "0") == "1":
        res = _run_sim(nc, in_maps)
    else:
        trace = bool(int(os.environ.get("KERNEL_TRACE", "0")))
        try:
            res = bass_utils.run_bass_kernel_spmd(
                nc, in_maps, core_ids=list(range(NCORES)), trace=trace)
        except ModuleNotFoundError:
            # NTFF profile hook unavailable in this environment — run untraced.
            os.environ["BASS_NEVER_TRACE"] = "1"
            res = bass_utils.run_bass_kernel_spmd(
                nc, in_maps, core_ids=list(range(NCORES)), trace=False)
    last_results = res
    last_exec_time_ns = res.exec_time_ns

    outs = []
    for c in range(NCORES):
        o = res.results[c]["out"].reshape(128, TPC, H)
        outs.append(o.transpose(1, 0, 2).reshape(ECP, H)[:EC])
    return np.ascontiguousarray(np.concatenate(outs, axis=0))
